# revision 54
# baseline (speedup 1.0000x reference)
"""Multi-head linear attention on Trainium2 — 8-core SPMD, batch+head sharded.

Full-tensor contract: kernel(**inputs) takes the complete Q/K/V
[4, 4096, 1024] f32 arrays, internally shards them across 8 NeuronCores
(core c -> batch c//2, heads 8*(c%2) .. 8*(c%2)+8, i.e. a contiguous
512-column slice of the embedding dim), runs one Bass kernel per core,
and reassembles the full [4, 4096, 1024] f32 output.

Per-core math (H=8 local heads, D=64, L=4096):
    phi = sigmoid(0.6053*x - 4.102)
    kv_ext[h] = phi_K[h]^T @ [V[h] | 1]     # [64, 65], f32 PSUM accum
    numden[h] = phi_Q[h] @ kv_ext[h]        # [L, 65]
    out[h]    = numden[h][:, :64] / numden[h][:, 64:65]

All device I/O is fp16 (host casts f32 -> fp16 in, fp16 -> f32 out;
matmul accumulation stays f32 in PSUM): 12.6 MiB loads + 4 MiB stores
per core.

The 8 heads form 4 PAIRS, processed as a 4-deep pipeline: pair g's
kv-accumulation streams while pair g-1's Q phase computes.  K and V are
MERGED row-wise on the host into one tensor of
[K_pair(128) | V_pair(128) | 1 | 1] rows, pre-permuted into exact SBUF
tile order so every DMA is a fully sequential sweep with maximal
(4128 B) descriptors.  One [128,130] matmul per 128-row chunk
accumulates kv AND k_sum with no wasted columns thanks to the baked-in
ones column.  The device is at its HBM roofline (8 cores share the
chip's ~2.9 TB/s), so the four KV batches of each pair are spread
across all three DMA dispatch queues (Pool/Sync/Scalar) to pull the
full per-core aggregate (~330 GB/s); phi_Q for a whole pair is
precomputed into resident SBUF as soon as its Q slice lands (1 KiB
descriptors so Q can't out-compete KV in round-robin), making Q-phase
pieces pure PE -> VectorE -> store chains; O stores alternate between
the Sync and Scalar queues, dispatched one piece late so their
data-waits never stall an engine stream.

kv for a pair accumulates in one [128, 130] PSUM tile (head0 rows 0:64,
head1 rows 64:128, k_sum in col 128); the Q-phase matmul multiplies a
128-q block of phi_Q^T against a block-diagonal [128, 130] kv operand,
yielding both heads' num|den.  Division runs on VectorE batched 3
q-blocks per PSUM bank: one strided reciprocal + one 4-D-strided
broadcast multiply.
"""

import numpy as np

B = 4
L = 4096
E = 1024
NH = 8            # heads per core
D = 64
W = D + 1         # head block width incl. ones/den column
EC = NH * D       # 512 embedding columns per core
P = 128
G = 4             # head pairs, stacked along rows (pipeline depth)
GC = EC // G      # 128 columns per pair
SUB = 2           # L-rows per partition line (1032 B descriptors)
VW = 2 * W        # 130: V|1|1 block width in KV staging / kv tiles
KVW = GC + VW     # 258: merged K|V|1|1 staged row width
NT = L // (P * SUB)   # 16 tiles (256 L-rows) per pair
TBS = 4           # tiles per DMA batch
NBS = NT // TBS   # 4 batches per pair
QB = 2048         # q columns per Q-phase piece
NQB = L // QB     # 2 pieces per pair
N_CORES = 8

_CACHE = {}


def _build_nc():
    from contextlib import ExitStack

    import concourse.bacc as bacc
    import concourse.bass as bass
    import concourse.mybir as mybir
    import concourse.tile as tile

    f32 = mybir.dt.float32
    f16 = mybir.dt.float16
    SIG = mybir.ActivationFunctionType.Sigmoid

    nc = bacc.Bacc("TRN2", target_bir_lowering=False, debug=False)
    # All DRAM tensors are staged by the host in exact SBUF-tile order so
    # every DMA is a fully sequential sweep with maximal descriptors: KV
    # batch (g, ib) occupies rows [(g*NBS+ib)*P, +P) as [p][t][s][e]
    # (4128 B descriptors); O piece (g, qb) occupies rows [(g*NQB+qb)*P,
    # +P) as [p][qk*P+e] (4 KiB descriptors).
    Q = nc.dram_tensor("Q", [EC, L], f16, kind="ExternalInput").ap()
    KV = nc.dram_tensor("KV", [G * NBS * P, TBS * SUB * KVW], f16,
                        kind="ExternalInput").ap()
    O = nc.dram_tensor("O", [G * NQB * P, QB], f16, kind="ExternalOutput").ap()

    with tile.TileContext(nc) as tc, ExitStack() as ctx:
        singles = ctx.enter_context(tc.tile_pool(name="singles", bufs=1))
        ld = ctx.enter_context(tc.tile_pool(name="ld", bufs=4))
        lds = ctx.enter_context(tc.tile_pool(name="lds", bufs=4))
        ldp = ctx.enter_context(tc.tile_pool(name="ldp", bufs=4))
        ph = ctx.enter_context(tc.tile_pool(name="ph", bufs=3))
        rcp = ctx.enter_context(tc.tile_pool(name="rcp", bufs=6))
        ob = ctx.enter_context(tc.tile_pool(name="ob", bufs=8))
        pn = ctx.enter_context(tc.tile_pool(name="pn", bufs=4, space="PSUM"))
        pk = ctx.enter_context(tc.tile_pool(name="pk", bufs=1, space="PSUM"))

        sig_bias = singles.tile([P, 1], f32)
        nc.vector.memset(sig_bias, -4.102)

        # Block-diagonal kv operand per head pair: rows 0:64 cols 0:65 hold
        # kv_ext of the even head, rows 64:128 cols 65:130 the odd head.
        kv_bd = singles.tile([P, G, VW], f16)
        nc.vector.memset(kv_bd, 0.0)

        # Full-bank PSUM tiles (no matmul output may straddle a bank).
        kv_ps = [pk.tile([P, 512], f32, tag=f"kv{g}", name=f"kv{g}")
                 for g in range(G)]

        # Whole-pair raw-Q and phi_Q resident buffers (8 KiB/part each).
        # q_raw keeps a 520-element row pitch: the 8-element gap stops the
        # DGE from coalescing adjacent 1 KiB runs into giant descriptors,
        # so Q can't out-compete the KV queues in DMA round-robin.
        q_raw = [singles.tile([P, 8, 520], f16, tag=f"qr{g}", name=f"qr{g}")
                 for g in range(G)]
        phiq = [singles.tile([P, L], f16, tag=f"pq{g}", name=f"pq{g}")
                for g in range(G)]

        def emit_q_load(g):
            nc.gpsimd.dma_start(
                out=q_raw[g][:, :, 0:512],
                in_=Q[g * P:(g + 1) * P, :].rearrange(
                    "p (k e) -> p k e", e=512),
            )

        def emit_q_sigmoid(g):
            nc.scalar.activation(
                out=phiq[g].rearrange("p (k e) -> p k e", e=512),
                in_=q_raw[g][:, :, 0:512], func=SIG, bias=sig_bias,
                scale=0.6053,
            )

        # KV batches are spread across all three queues so the stream gets
        # the full aggregate bandwidth: ib0 front-loaded on the Pool queue
        # (interleaved with Q loads), ib1/ib3 on the Scalar queue hoisted
        # one phase ahead of their sigmoid consumers, ib2 in-phase on
        # Sync (pair 0, needed immediately, keeps ib0/ib2/ib3 on Sync and
        # ib1 on Scalar — the fast-starting queues).
        kv_tiles = {}

        def emit_kv_dispatch(g, ib, engine, pool):
            rows = slice((g * NBS + ib) * P, (g * NBS + ib + 1) * P)
            kvt = pool.tile([P, TBS, SUB, KVW], f16, tag=f"kvt{ib}",
                            name=f"kvt{ib}")
            engine.dma_start(
                out=kvt,
                in_=KV[rows, :].rearrange("p (t s e) -> p t s e",
                                          t=TBS, s=SUB),
            )
            kv_tiles[(g, ib)] = kvt

        def emit_kv_compute(g, ib):
            kvt = kv_tiles.pop((g, ib))
            phiK = ph.tile([P, TBS, SUB, GC], f16, tag="phiK", name="phiK")
            nc.scalar.activation(
                out=phiK, in_=kvt[:, :, :, 0:GC], func=SIG, bias=sig_bias,
                scale=0.6053,
            )
            for t in range(TBS):
                for s in range(SUB):
                    nc.tensor.matmul(
                        out=kv_ps[g][:, 0:VW],
                        lhsT=phiK[:, t, s, :],
                        rhs=kvt[:, t, s, GC:KVW],
                        start=(ib == 0 and t == 0 and s == 0),
                        stop=(ib == NBS - 1 and t == TBS - 1
                              and s == SUB - 1),
                    )

        def emit_kv_finish(g):
            """Pack the pair's kv PSUM tile into the block-diag operand."""
            nc.vector.tensor_copy(
                out=kv_bd[0:D, g, 0:D], in_=kv_ps[g][0:D, 0:D])
            nc.vector.tensor_copy(
                out=kv_bd[0:D, g, D:W], in_=kv_ps[g][0:D, 2 * D:2 * D + 1])
            nc.vector.tensor_copy(
                out=kv_bd[D:P, g, W:W + D], in_=kv_ps[g][D:P, D:2 * D])
            nc.vector.tensor_copy(
                out=kv_bd[D:P, g, W + D:VW], in_=kv_ps[g][D:P, 2 * D:2 * D + 1])

        # O-store dispatches alternate between the Sync and Pool queues
        # but are ALL held until after the last load dispatch is emitted:
        # each queue's FIFO then serves every load descriptor before any
        # store descriptor, so stores can't steal HBM share from the
        # loads that gate the pipeline — they drain into the tail where
        # DMA is otherwise idle.  (Scalar's engine is busy with sigmoids
        # until late, so it gets no stores.)
        pending_stores = []
        store_rr = [0]

        def flush_stores():
            while pending_stores:
                pending_stores.pop(0)()

        def emit_q_piece(g, qb, store_chunks=1):
            """16 matmuls + batched div + store(s) (no Scalar dep).

            store_chunks > 1 splits the store so the final pieces' output
            drains while later q-blocks are still computing."""
            out_t = ob.tile([P, QB], f16, tag="outt", name="out_t")
            nqk = QB // P               # 16
            per_chunk = nqk // store_chunks
            orow = (g * NQB + qb) * P

            def store(c0, c1):
                eng = (nc.sync, nc.gpsimd)[store_rr[0] % 2]
                store_rr[0] += 1
                eng.dma_start(
                    out=O[orow:orow + P, c0 * P:c1 * P],
                    in_=out_t[:, c0 * P:c1 * P],
                )

            qk = 0
            next_store = per_chunk
            while qk < nqk:
                nb = min(3, nqk - qk, next_store - qk)
                num = pn.tile([P, 3, VW], f32, tag="num", name="num")
                for i in range(nb):
                    nc.tensor.matmul(
                        out=num[:, i, :],
                        lhsT=phiq[g][:, qb * QB + (qk + i) * P:
                                     qb * QB + (qk + i + 1) * P],
                        rhs=kv_bd[:, g, :],
                    )
                r = rcp.tile([P, 3, 2], f32, tag="r", name="r")
                den = bass.AP(
                    tensor=num.tensor, offset=num.offset + D,
                    ap=[num.ap[0], [VW, nb], [W, 2]],
                )
                nc.vector.reciprocal(out=r[:, 0:nb, :], in_=den)
                nums = bass.AP(
                    tensor=num.tensor, offset=num.offset,
                    ap=[num.ap[0], [VW, nb], [W, 2], [1, D]],
                )
                r_bc = bass.AP(
                    tensor=r.tensor, offset=r.offset,
                    ap=[r.ap[0], [2, nb], [1, 2], [0, D]],
                )
                nc.vector.tensor_tensor(
                    out=out_t[:, qk * P:(qk + nb) * P].rearrange(
                        "p (a b d) -> p a b d", a=nb, b=2),
                    in0=nums, in1=r_bc, op=mybir.AluOpType.mult,
                )
                qk += nb
                if qk == next_store:
                    c0, c1 = next_store - per_chunk, next_store
                    if store_chunks > 1:
                        store(c0, c1)
                    else:
                        pending_stores.append(
                            lambda c0=c0, c1=c1: store(c0, c1))
                    next_store += per_chunk

        # ---- software-pipelined emission: pair g's kv streams while
        # pair g-1's Q phase computes. ----
        def emit_scalar_dispatch(g):
            if g < G:
                emit_kv_dispatch(g, 1, nc.scalar, lds)
                if g > 0:
                    emit_kv_dispatch(g, 3, nc.scalar, lds)

        # Pool stream: Q0 first (it gates pair 0's Q phase), then later
        # pairs' ib0 front-loads interleaved with the remaining Q loads.
        # Pair 0 avoids the slow-starting Pool queue entirely.
        emit_q_load(0)
        for g in range(1, G):
            emit_kv_dispatch(g, 0, nc.gpsimd, ldp)
            emit_q_load(g)
        emit_scalar_dispatch(0)
        emit_scalar_dispatch(1)
        for ib in (0, 2, 3):
            emit_kv_dispatch(0, ib, nc.sync, ld)
        for ib in range(NBS):
            emit_kv_compute(0, ib)
        emit_q_sigmoid(0)
        emit_kv_finish(0)
        for g in range(1, G):
            emit_kv_dispatch(g, 2, nc.sync, ld)
            emit_scalar_dispatch(g + 1)
            for ib in range(NBS):
                emit_kv_compute(g, ib)
                if ib % 2 == 1:
                    emit_q_piece(g - 1, (ib - 1) // 2)
            emit_q_sigmoid(g)
            emit_kv_finish(g)
        # every load dispatch is emitted by now — release the store
        # dispatches so their descriptors queue strictly behind the loads
        flush_stores()
        for qb in range(NQB):
            emit_q_piece(G - 1, qb, store_chunks=4)
        flush_stores()

    nc.compile()
    return nc


def _get_nc():
    if "nc" not in _CACHE:
        _CACHE["nc"] = _build_nc()
    return _CACHE["nc"]


def _shard_q(arr):
    """Full [B, L, E] f32 -> per-core transposed [512, L] fp16 slices."""
    out = []
    for c in range(N_CORES):
        b, g = divmod(c, 2)
        out.append(np.ascontiguousarray(
            arr[b, :, g * EC:(g + 1) * EC].T.astype(np.float16)))
    return out


def _shard_kv(karr, varr):
    """Full K/V [B, L, E] f32 -> per-core [4*4*128, 2064] fp16: merged
    [K_pair(128) | V_pair(128) | 1 | 1] rows pre-permuted into SBUF tile
    order [g][ib][p][t][s][e] so every batch DMA is one sequential
    sweep."""
    out = []
    for c in range(N_CORES):
        b, g = divmod(c, 2)
        ksl = karr[b, :, g * EC:(g + 1) * EC].astype(np.float16)
        vsl = varr[b, :, g * EC:(g + 1) * EC].astype(np.float16)
        st = np.ones((G, L, KVW), dtype=np.float16)
        for pg in range(G):
            st[pg, :, 0:GC] = ksl[:, pg * GC:(pg + 1) * GC]
            st[pg, :, GC:GC + P] = vsl[:, pg * P:(pg + 1) * P]
        # row (g, ib, t, p, s) -> staged position (g, ib, p, t, s)
        perm = st.reshape(G, NBS, TBS, P, SUB, KVW).transpose(0, 1, 3, 2, 4, 5)
        out.append(np.ascontiguousarray(
            perm.reshape(G * NBS * P, TBS * SUB * KVW)))
    return out


def _unshard_o(o):
    """Per-core [4*2*128, 2048] fp16 (piece-major) -> [L, EC] f32 slice."""
    blocks = o.reshape(G, NQB, P, QB // P, P)   # [pg, qb, p, qk, e]
    # q = qb*QB + qk*P + p
    perm = blocks.transpose(0, 1, 3, 2, 4).reshape(G, L, P)
    return np.concatenate(list(perm), axis=1).astype(np.float32)


def run_sharded(in_maps, trace=False, trace_cores=None):
    from concourse.bass_utils import run_bass_kernel_spmd

    nc = _get_nc()
    kwargs = {}
    if trace:
        kwargs = dict(trace=True, trace_cores=trace_cores or [0])
    return run_bass_kernel_spmd(nc, in_maps, core_ids=list(range(N_CORES)), **kwargs)


def kernel(**inputs):
    Q = np.asarray(inputs["Q"], dtype=np.float32)
    K = np.asarray(inputs["K"], dtype=np.float32)
    V = np.asarray(inputs["V"], dtype=np.float32)
    qs, kvs = _shard_q(Q), _shard_kv(K, V)
    in_maps = [{"Q": qs[c], "KV": kvs[c]} for c in range(N_CORES)]
    res = run_sharded(in_maps)
    out = np.empty((B, L, E), dtype=np.float32)
    for c in range(N_CORES):
        b, g = divmod(c, 2)
        out[b, :, g * EC:(g + 1) * EC] = _unshard_o(res.results[c]["O"])
    return out


# revision 55
# speedup vs baseline: 1.0190x; 1.0190x over previous
"""Multi-head linear attention on Trainium2 — 8-core SPMD, batch+head sharded.

Full-tensor contract: kernel(**inputs) takes the complete Q/K/V
[4, 4096, 1024] f32 arrays, internally shards them across 8 NeuronCores
(core c -> batch c//2, heads 8*(c%2) .. 8*(c%2)+8, i.e. a contiguous
512-column slice of the embedding dim), runs one Bass kernel per core,
and reassembles the full [4, 4096, 1024] f32 output.

Per-core math (H=8 local heads, D=64, L=4096):
    phi = sigmoid(0.6053*x - 4.102)
    kv_ext[h] = phi_K[h]^T @ [V[h] | 1]     # [64, 65], f32 PSUM accum
    numden[h] = phi_Q[h] @ kv_ext[h]        # [L, 65]
    out[h]    = numden[h][:, :64] / numden[h][:, 64:65]

All device I/O is fp16 (host casts f32 -> fp16 in, fp16 -> f32 out;
matmul accumulation stays f32 in PSUM): 12.6 MiB loads + 4 MiB stores
per core.

The 8 heads form 4 PAIRS, processed as a 4-deep pipeline: pair g's
kv-accumulation streams while pair g-1's Q phase computes.  K and V are
MERGED row-wise on the host into one tensor of
[K_pair(128) | V_pair(128) | 1 | 1] rows, pre-permuted into exact SBUF
tile order so every DMA is a fully sequential sweep with maximal
(4128 B) descriptors.  One [128,130] matmul per 128-row chunk
accumulates kv AND k_sum with no wasted columns thanks to the baked-in
ones column.  The device is at its HBM roofline (8 cores share the
chip's ~2.9 TB/s), so the four KV batches of each pair are spread
across all three DMA dispatch queues (Pool/Sync/Scalar) to pull the
full per-core aggregate (~330 GB/s); phi_Q for a whole pair is
precomputed into resident SBUF as soon as its Q slice lands (1 KiB
descriptors so Q can't out-compete KV in round-robin), making Q-phase
pieces pure PE -> VectorE -> store chains; O stores alternate between
the Sync and Scalar queues, dispatched one piece late so their
data-waits never stall an engine stream.

kv for a pair accumulates in one [128, 130] PSUM tile (head0 rows 0:64,
head1 rows 64:128, k_sum in col 128); the Q-phase matmul multiplies a
128-q block of phi_Q^T against a block-diagonal [128, 130] kv operand,
yielding both heads' num|den.  Division runs on VectorE batched 3
q-blocks per PSUM bank: one strided reciprocal + one 4-D-strided
broadcast multiply.
"""

import numpy as np

B = 4
L = 4096
E = 1024
NH = 8            # heads per core
D = 64
W = D + 1         # head block width incl. ones/den column
EC = NH * D       # 512 embedding columns per core
P = 128
G = 4             # head pairs, stacked along rows (pipeline depth)
GC = EC // G      # 128 columns per pair
SUB = 2           # L-rows per partition line (1032 B descriptors)
VW = 2 * W        # 130: V|1|1 block width in KV staging / kv tiles
KVW = GC + VW     # 258: merged K|V|1|1 staged row width
NT = L // (P * SUB)   # 16 tiles (256 L-rows) per pair
TBS = 4           # tiles per DMA batch
NBS = NT // TBS   # 4 batches per pair
QB = 2048         # q columns per Q-phase piece
NQB = L // QB     # 2 pieces per pair
N_CORES = 8

_CACHE = {}


def _build_nc():
    from contextlib import ExitStack

    import concourse.bacc as bacc
    import concourse.bass as bass
    import concourse.mybir as mybir
    import concourse.tile as tile

    f32 = mybir.dt.float32
    f16 = mybir.dt.float16
    SIG = mybir.ActivationFunctionType.Sigmoid

    nc = bacc.Bacc("TRN2", target_bir_lowering=False, debug=False)
    # All DRAM tensors are staged by the host in exact SBUF-tile order so
    # every DMA is a fully sequential sweep with maximal descriptors: KV
    # batch (g, ib) occupies rows [(g*NBS+ib)*P, +P) as [p][t][s][e]
    # (4128 B descriptors); O piece (g, qb) occupies rows [(g*NQB+qb)*P,
    # +P) as [p][qk*P+e] (4 KiB descriptors).
    Q = nc.dram_tensor("Q", [EC, L], f16, kind="ExternalInput").ap()
    KV = nc.dram_tensor("KV", [G * NBS * P, TBS * SUB * KVW], f16,
                        kind="ExternalInput").ap()
    O = nc.dram_tensor("O", [G * NQB * P, QB], f16, kind="ExternalOutput").ap()

    with tile.TileContext(nc) as tc, ExitStack() as ctx:
        singles = ctx.enter_context(tc.tile_pool(name="singles", bufs=1))
        ld = ctx.enter_context(tc.tile_pool(name="ld", bufs=4))
        lds = ctx.enter_context(tc.tile_pool(name="lds", bufs=4))
        ldp = ctx.enter_context(tc.tile_pool(name="ldp", bufs=4))
        ph = ctx.enter_context(tc.tile_pool(name="ph", bufs=3))
        rcp = ctx.enter_context(tc.tile_pool(name="rcp", bufs=6))
        ob = ctx.enter_context(tc.tile_pool(name="ob", bufs=5))
        pn = ctx.enter_context(tc.tile_pool(name="pn", bufs=4, space="PSUM"))
        pk = ctx.enter_context(tc.tile_pool(name="pk", bufs=1, space="PSUM"))

        sig_bias = singles.tile([P, 1], f32)
        nc.vector.memset(sig_bias, -4.102)

        # Block-diagonal kv operand per head pair: rows 0:64 cols 0:65 hold
        # kv_ext of the even head, rows 64:128 cols 65:130 the odd head.
        kv_bd = singles.tile([P, G, VW], f16)
        nc.vector.memset(kv_bd, 0.0)

        # Full-bank PSUM tiles (no matmul output may straddle a bank).
        kv_ps = [pk.tile([P, 512], f32, tag=f"kv{g}", name=f"kv{g}")
                 for g in range(G)]

        # Whole-pair raw-Q and phi_Q resident buffers (8 KiB/part each).
        # q_raw keeps a 520-element row pitch: the 8-element gap stops the
        # DGE from coalescing adjacent 1 KiB runs into giant descriptors,
        # so Q can't out-compete the KV queues in DMA round-robin.
        q_raw = [singles.tile([P, 8, 520], f16, tag=f"qr{g}", name=f"qr{g}")
                 for g in range(G)]
        phiq = [singles.tile([P, L], f16, tag=f"pq{g}", name=f"pq{g}")
                for g in range(G)]

        def emit_q_load(g):
            nc.gpsimd.dma_start(
                out=q_raw[g][:, :, 0:512],
                in_=Q[g * P:(g + 1) * P, :].rearrange(
                    "p (k e) -> p k e", e=512),
            )

        def emit_q_sigmoid(g):
            nc.scalar.activation(
                out=phiq[g].rearrange("p (k e) -> p k e", e=512),
                in_=q_raw[g][:, :, 0:512], func=SIG, bias=sig_bias,
                scale=0.6053,
            )

        # KV batches are spread across all three queues so the stream gets
        # the full aggregate bandwidth: ib0 front-loaded on the Pool queue
        # (interleaved with Q loads), ib1/ib3 on the Scalar queue hoisted
        # one phase ahead of their sigmoid consumers, ib2 in-phase on
        # Sync (pair 0, needed immediately, keeps ib0/ib2/ib3 on Sync and
        # ib1 on Scalar — the fast-starting queues).
        kv_tiles = {}

        def emit_kv_dispatch(g, ib, engine, pool):
            rows = slice((g * NBS + ib) * P, (g * NBS + ib + 1) * P)
            kvt = pool.tile([P, TBS, SUB, KVW], f16, tag=f"kvt{ib}",
                            name=f"kvt{ib}")
            engine.dma_start(
                out=kvt,
                in_=KV[rows, :].rearrange("p (t s e) -> p t s e",
                                          t=TBS, s=SUB),
            )
            kv_tiles[(g, ib)] = kvt

        def emit_kv_compute(g, ib):
            kvt = kv_tiles.pop((g, ib))
            phiK = ph.tile([P, TBS, SUB, GC], f16, tag="phiK", name="phiK")
            nc.scalar.activation(
                out=phiK, in_=kvt[:, :, :, 0:GC], func=SIG, bias=sig_bias,
                scale=0.6053,
            )
            for t in range(TBS):
                for s in range(SUB):
                    nc.tensor.matmul(
                        out=kv_ps[g][:, 0:VW],
                        lhsT=phiK[:, t, s, :],
                        rhs=kvt[:, t, s, GC:KVW],
                        start=(ib == 0 and t == 0 and s == 0),
                        stop=(ib == NBS - 1 and t == TBS - 1
                              and s == SUB - 1),
                    )

        def emit_kv_finish(g):
            """Pack the pair's kv PSUM tile into the block-diag operand."""
            nc.vector.tensor_copy(
                out=kv_bd[0:D, g, 0:D], in_=kv_ps[g][0:D, 0:D])
            nc.vector.tensor_copy(
                out=kv_bd[0:D, g, D:W], in_=kv_ps[g][0:D, 2 * D:2 * D + 1])
            nc.vector.tensor_copy(
                out=kv_bd[D:P, g, W:W + D], in_=kv_ps[g][D:P, D:2 * D])
            nc.vector.tensor_copy(
                out=kv_bd[D:P, g, W + D:VW], in_=kv_ps[g][D:P, 2 * D:2 * D + 1])

        # O-store dispatches alternate between the Sync and Scalar queues
        # (both drain out after the loads, so the 4 MiB O stream gets two
        # queues' bandwidth) and are DEFERRED one piece: by dispatch time
        # the data is long complete, so a dispatch never stalls its
        # engine stream behind a data-wait.
        pending_stores = []
        store_rr = [0]

        def flush_stores():
            while pending_stores:
                pending_stores.pop(0)()

        def emit_q_piece(g, qb, store_chunks=1):
            """16 matmuls + batched div + store(s) (no Scalar dep).

            store_chunks > 1 splits the store so the final pieces' output
            drains while later q-blocks are still computing."""
            flush_stores()
            out_t = ob.tile([P, QB], f16, tag="outt", name="out_t")
            nqk = QB // P               # 16
            per_chunk = nqk // store_chunks
            orow = (g * NQB + qb) * P

            def store(c0, c1):
                eng = (nc.sync, nc.scalar)[store_rr[0] % 2]
                store_rr[0] += 1
                eng.dma_start(
                    out=O[orow:orow + P, c0 * P:c1 * P],
                    in_=out_t[:, c0 * P:c1 * P],
                )

            qk = 0
            next_store = per_chunk
            while qk < nqk:
                nb = min(3, nqk - qk, next_store - qk)
                num = pn.tile([P, 3, VW], f32, tag="num", name="num")
                for i in range(nb):
                    nc.tensor.matmul(
                        out=num[:, i, :],
                        lhsT=phiq[g][:, qb * QB + (qk + i) * P:
                                     qb * QB + (qk + i + 1) * P],
                        rhs=kv_bd[:, g, :],
                    )
                r = rcp.tile([P, 3, 2], f32, tag="r", name="r")
                den = bass.AP(
                    tensor=num.tensor, offset=num.offset + D,
                    ap=[num.ap[0], [VW, nb], [W, 2]],
                )
                nc.vector.reciprocal(out=r[:, 0:nb, :], in_=den)
                nums = bass.AP(
                    tensor=num.tensor, offset=num.offset,
                    ap=[num.ap[0], [VW, nb], [W, 2], [1, D]],
                )
                r_bc = bass.AP(
                    tensor=r.tensor, offset=r.offset,
                    ap=[r.ap[0], [2, nb], [1, 2], [0, D]],
                )
                nc.vector.tensor_tensor(
                    out=out_t[:, qk * P:(qk + nb) * P].rearrange(
                        "p (a b d) -> p a b d", a=nb, b=2),
                    in0=nums, in1=r_bc, op=mybir.AluOpType.mult,
                )
                qk += nb
                if qk == next_store:
                    c0, c1 = next_store - per_chunk, next_store
                    if store_chunks > 1:
                        store(c0, c1)
                    else:
                        pending_stores.append(
                            lambda c0=c0, c1=c1: store(c0, c1))
                    next_store += per_chunk

        # ---- software-pipelined emission: pair g's kv streams while
        # pair g-1's Q phase computes. ----
        def emit_scalar_dispatch(g):
            if g < G:
                emit_kv_dispatch(g, 1, nc.scalar, lds)
                if g > 0:
                    emit_kv_dispatch(g, 3, nc.scalar, lds)

        # Pool stream: Q0 first (it gates pair 0's Q phase), then later
        # pairs' ib0 front-loads interleaved with the remaining Q loads.
        # Pair 0 avoids the slow-starting Pool queue entirely.
        emit_q_load(0)
        for g in range(1, G):
            emit_kv_dispatch(g, 0, nc.gpsimd, ldp)
            emit_q_load(g)
        emit_scalar_dispatch(0)
        emit_scalar_dispatch(1)
        for ib in (0, 2, 3):
            emit_kv_dispatch(0, ib, nc.sync, ld)
        for ib in range(NBS):
            emit_kv_compute(0, ib)
        emit_q_sigmoid(0)
        emit_kv_finish(0)
        for g in range(1, G):
            emit_kv_dispatch(g, 2, nc.sync, ld)
            emit_scalar_dispatch(g + 1)
            for ib in range(NBS):
                emit_kv_compute(g, ib)
                if ib % 2 == 1:
                    emit_q_piece(g - 1, (ib - 1) // 2)
            emit_q_sigmoid(g)
            emit_kv_finish(g)
        for qb in range(NQB):
            emit_q_piece(G - 1, qb, store_chunks=4)
        flush_stores()

    nc.compile()
    return nc


def _get_nc():
    if "nc" not in _CACHE:
        _CACHE["nc"] = _build_nc()
    return _CACHE["nc"]


def _shard_q(arr):
    """Full [B, L, E] f32 -> per-core transposed [512, L] fp16 slices."""
    out = []
    for c in range(N_CORES):
        b, g = divmod(c, 2)
        out.append(np.ascontiguousarray(
            arr[b, :, g * EC:(g + 1) * EC].T.astype(np.float16)))
    return out


def _shard_kv(karr, varr):
    """Full K/V [B, L, E] f32 -> per-core [4*4*128, 2064] fp16: merged
    [K_pair(128) | V_pair(128) | 1 | 1] rows pre-permuted into SBUF tile
    order [g][ib][p][t][s][e] so every batch DMA is one sequential
    sweep."""
    out = []
    for c in range(N_CORES):
        b, g = divmod(c, 2)
        ksl = karr[b, :, g * EC:(g + 1) * EC].astype(np.float16)
        vsl = varr[b, :, g * EC:(g + 1) * EC].astype(np.float16)
        st = np.ones((G, L, KVW), dtype=np.float16)
        for pg in range(G):
            st[pg, :, 0:GC] = ksl[:, pg * GC:(pg + 1) * GC]
            st[pg, :, GC:GC + P] = vsl[:, pg * P:(pg + 1) * P]
        # row (g, ib, t, p, s) -> staged position (g, ib, p, t, s)
        perm = st.reshape(G, NBS, TBS, P, SUB, KVW).transpose(0, 1, 3, 2, 4, 5)
        out.append(np.ascontiguousarray(
            perm.reshape(G * NBS * P, TBS * SUB * KVW)))
    return out


def _unshard_o(o):
    """Per-core [4*2*128, 2048] fp16 (piece-major) -> [L, EC] f32 slice."""
    blocks = o.reshape(G, NQB, P, QB // P, P)   # [pg, qb, p, qk, e]
    # q = qb*QB + qk*P + p
    perm = blocks.transpose(0, 1, 3, 2, 4).reshape(G, L, P)
    return np.concatenate(list(perm), axis=1).astype(np.float32)


def run_sharded(in_maps, trace=False, trace_cores=None):
    from concourse.bass_utils import run_bass_kernel_spmd

    nc = _get_nc()
    kwargs = {}
    if trace:
        kwargs = dict(trace=True, trace_cores=trace_cores or [0])
    return run_bass_kernel_spmd(nc, in_maps, core_ids=list(range(N_CORES)), **kwargs)


def kernel(**inputs):
    Q = np.asarray(inputs["Q"], dtype=np.float32)
    K = np.asarray(inputs["K"], dtype=np.float32)
    V = np.asarray(inputs["V"], dtype=np.float32)
    qs, kvs = _shard_q(Q), _shard_kv(K, V)
    in_maps = [{"Q": qs[c], "KV": kvs[c]} for c in range(N_CORES)]
    res = run_sharded(in_maps)
    out = np.empty((B, L, E), dtype=np.float32)
    for c in range(N_CORES):
        b, g = divmod(c, 2)
        out[b, :, g * EC:(g + 1) * EC] = _unshard_o(res.results[c]["O"])
    return out
